# revision 18
# baseline (speedup 1.0000x reference)
"""Trainium2 Bass kernel: gradient of the EnergyAttention scalar energy.

reference:
    q = einsum('bqd,hzd->bqhz', g, wq); k = einsum('bkd,hzd->bkhz', g, wk)
    scores = einsum('bqhz,bkhz->bhqk', q, k)
    E = -(logsumexp(BETA*scores, -1)/BETA).sum() + POS_SCALE*(g*pos).sum()
    out = dE/dg

Math: with P = softmax(BETA*scores) per (b,h,q):
    out[b] = -sum_h [ (P@K) @ wq_h + (P.T@Qn) @ wk_h ] + POS_SCALE*pos
where Qn = diag(1/Z) Q (row-normalized by the softmax partition Z).

Sharding: 8 cores; core c handles batch b=c//4 and heads 4*(c%4)..4*(c%4)+3
(two head-pairs packed into the 128-partition dim).  Each core UPLOADS only
its 256-row chunk of x (bf16) plus its own 4 heads of wq/wk (bf16, natural
layout); x is AllGather'd on-device inside each 4-core batch group, and the
positive partials are ReduceScatter'd on-device so each core DOWNLOADS only
its 256 rows of the final output (negation + positional term fused on DVE).

Per-core structure (fused pipeline; P tiles are consumed by the transposed
projection matmuls as soon as they are exp'd, so no full [S,S] matrix is
ever materialized in f32):
  comm:   DRAM AllGather of x chunks -> xg [S, D] (own batch)
  prep:   PE transposes xg -> G^T tiles; wq/wk natural -> W^T tiles
  proj:   QT2/KT2 [z2, s] = (W G^T) via bf16 matmuls (d contracted in 8 tiles)
  trans:  Qraw/K2n [s, z2] via PE transpose-mode
  loop i: scores blocks (row-tiled K=64 pairs, bf16) -> exp on ACT (fp16 out,
          fused row-sum accum for Z) -> P^T blocks via PE transpose (replaces
          the transposed-scores recompute AND its exp)
          -> dK^T += (Q/Z)^T-block @ P-block  (col-tiled M=64 pairs, fp16)
  burst:  dQ^T += K-block @ PT-block (deferred one pair for overlap)
  out:    grad = sum_pairs dQT^T wq + dKT^T wk  (bf16 matmuls)
  comm:   2x ReduceScatter(add) over the 4-core batch group; fused
          out = pos - acc on DVE; each core writes its [256, D] chunk.
"""

import numpy as np

B = 2
S = 1024
D = 1024
NH = 16
Z = 64
BETA = 1.0 / np.sqrt(np.float32(Z))
POS_SCALE = 0.001
N_CORES = 8
HPC = 4           # heads per core
NPAIR = 2         # head pairs per core
ND = D // 128     # 8 d-tiles
NQ = S // 128     # 8 q/k blocks
NCH = S // 512    # 2 moving-dim chunks
CHUNK = S // 4    # 256 rows of x uploaded / of out downloaded per core

_CACHE = {}


def build_nc(reps=1):
    """Build the (SPMD, identical-per-core) Bass program.

    reps>1 repeats the whole computation (idempotent) inside one NEFF --
    used to measure steady-state per-execution time as a marginal cost."""
    from contextlib import ExitStack

    import concourse.mybir as mybir
    import concourse.tile as tile
    from concourse import bacc
    from concourse.masks import make_identity

    F32 = mybir.dt.float32
    BF16 = mybir.dt.bfloat16
    F16 = mybir.dt.float16
    MUL = mybir.AluOpType.mult
    ADD = mybir.AluOpType.add
    BYP = mybir.AluOpType.bypass
    EXP = mybir.ActivationFunctionType.Exp
    CPY = mybir.ActivationFunctionType.Copy
    GROUPS = [[0, 1, 2, 3], [4, 5, 6, 7]]

    nc = bacc.Bacc(
        "TRN2",
        target_bir_lowering=False,
        debug=False,
        enable_asserts=False,
        num_devices=N_CORES,
    )

    xc = nc.dram_tensor("xc", [CHUNK, D], BF16, kind="ExternalInput").ap()
    wqn_in = nc.dram_tensor("wqn", [NPAIR * 128, D], BF16, kind="ExternalInput").ap()
    wkn_in = nc.dram_tensor("wkn", [NPAIR * 128, D], BF16, kind="ExternalInput").ap()
    posc = nc.dram_tensor("posc", [128, 2], F32, kind="ExternalInput").ap()
    gout = nc.dram_tensor("gout", [CHUNK, D], F32, kind="ExternalOutput").ap()

    with tile.TileContext(nc) as tc, ExitStack() as ctx:
        sb1 = ctx.enter_context(tc.tile_pool(name="sb1", bufs=1))
        sb2 = ctx.enter_context(tc.tile_pool(name="sb2", bufs=2))
        sb3 = ctx.enter_context(tc.tile_pool(name="sb3", bufs=3))
        sb4 = ctx.enter_context(tc.tile_pool(name="sb4", bufs=4))
        pp = ctx.enter_context(tc.tile_pool(name="pp", bufs=8))
        dram = ctx.enter_context(tc.tile_pool(name="dram", bufs=1, space="DRAM"))
        # PSUM: "sc" 2x[128,1024] (4 banks) shared by proj/transposes/scores/
        # P-transposes/outproj; "d" 2x[128,1024] (4 banks) for the dK then dQ
        # accumulators (each head's accumulator owns a whole tile so each
        # has_written group has its own banks) -> exactly 8 banks.
        # ps_sc tiles are all <= 1 PSUM bank ([128,512] f32 or [128,1024]
        # 16-bit) so 4 bufs fit in 4 banks and the scores->exp->transpose
        # pipeline runs several stages deep.
        ps_sc = ctx.enter_context(tc.tile_pool(name="ps_sc", bufs=4, space="PSUM"))
        ps_d = ctx.enter_context(tc.tile_pool(name="ps_d", bufs=2, space="PSUM"))

        ident = sb1.tile([128, 128], F32, tag="ident")
        make_identity(nc, ident[:])
        ident_b = sb1.tile([128, 128], BF16, tag="ident_b")
        nc.vector.tensor_copy(ident_b[:], ident[:])
        ident_h = sb1.tile([128, 128], F16, tag="ident_h")
        nc.vector.tensor_copy(ident_h[:], ident[:])

        for _rep in range(reps):
            # ---- x AllGather (own 4-core batch group) -----------------------
            xin_b = dram.tile([CHUNK, D], BF16, tag="xin_b", name=f"xin{_rep}")
            xg = dram.tile([S, D], BF16, tag="xg", name=f"xg{_rep}")
            nc.sync.dma_start(xin_b[:], xc[:])
            nc.gpsimd.collective_compute(
                "AllGather", BYP, replica_groups=GROUPS,
                ins=[xin_b[:]], outs=[xg[:]],
            )

            # ---- weights: natural load + W^T via PE transposes --------------
            # wnq/wnk [z2, (pair, d)] bf16 (outproj layout, direct load)
            wnq = sb1.tile([128, NPAIR * D], BF16, tag="wnq")
            wnk = sb1.tile([128, NPAIR * D], BF16, tag="wnk")
            for w_in, wn in ((wqn_in, wnq), (wkn_in, wnk)):
                nc.sync.dma_start(
                    wn[:].rearrange("p (b d) -> p b d", b=NPAIR),
                    w_in[:].rearrange("(b p) d -> p b d", p=128),
                )
            # wtq/wtk [d_in_tile, (pair, dt, z2)] bf16 via transposes
            wtq = sb1.tile([128, NPAIR * ND * 128], BF16, tag="wtq")
            wtk = sb1.tile([128, NPAIR * ND * 128], BF16, tag="wtk")
            for wn, wt in ((wnq, wtq), (wnk, wtk)):
                for p in range(NPAIR):
                    ps = ps_sc.tile([128, S], BF16, tag="ps_sc",
                                    name=f"wt{_rep}_{wt.tensor.name}_{p}")
                    for dt in range(ND):
                        nc.tensor.transpose(
                            ps[:, dt * 128 : (dt + 1) * 128],
                            wn[:, p * D + dt * 128 : p * D + (dt + 1) * 128],
                            ident_b[:],
                        )
                    nc.vector.tensor_copy(
                        wt[:, p * ND * 128 : (p + 1) * ND * 128], ps[:]
                    )

            # ---- G^T tiles from the gathered x via PE transposes ------------
            gt = sb1.tile([128, ND * S], BF16, tag="gt")  # [d_in_tile, (dt, s)]
            gt_r = gt[:].rearrange("p (d s) -> p d s", d=ND)
            for i in range(NQ):
                xs = sb3.tile([128, D], BF16, tag="xs", name=f"xs{_rep}_{i}")
                nc.sync.dma_start(xs[:], xg[i * 128 : (i + 1) * 128, :])
                ps = ps_sc.tile([128, S], BF16, tag="ps_sc", name=f"xt{_rep}_{i}")
                for dt in range(ND):
                    nc.tensor.transpose(
                        ps[:, dt * 128 : (dt + 1) * 128],
                        xs[:, dt * 128 : (dt + 1) * 128],
                        ident_b[:],
                    )
                nc.vector.tensor_copy(
                    gt_r[:, :, i * 128 : (i + 1) * 128],
                    ps[:].rearrange("p (d c) -> p d c", d=ND),
                )

            # persistent across pairs
            dqt2 = sb1.tile([128, NPAIR * S], BF16, tag="dqt2")  # [z2, (pair, q)]
            dkt2 = sb1.tile([128, NPAIR * S], BF16, tag="dkt2")  # [z2, (pair, k)]
            zrowA = sb1.tile([1, S], F32, tag="zrowA")
            zrowB = sb1.tile([1, S], F32, tag="zrowB")
            ztsb = sb1.tile([16, 128], F32, tag="ztsb")

            pending_dq = []

            def emit_dq_burst():
                """dQ^T(unnorm) += K_i^T PT_i over all blocks, then Z-scale."""
                if not pending_dq:
                    return
                PT_a, k2n_a, zbc_ab, pa = pending_dq.pop()
                dq_ps = [
                    ps_d.tile([128, S], F32, tag="ps_d", name=f"dq_ps{pa}_{a}")
                    for a in range(2)
                ]
                for i in range(NQ):
                    for a in range(2):
                        for ch in range(NCH):
                            nc.tensor.matmul(
                                dq_ps[a][a * 64 : (a + 1) * 64, ch * 512 : (ch + 1) * 512],
                                lhsT=k2n_a[:, i * 128 + a * 64 : i * 128 + (a + 1) * 64],
                                rhs=PT_a[:, (a * NQ + i) * S + ch * 512 : (a * NQ + i) * S + ch * 512 + 512],
                                start=(i == 0),
                                stop=(i == NQ - 1),
                            )
                for a in range(2):
                    nc.vector.tensor_tensor(
                        dqt2[a * 64 : (a + 1) * 64, pa * S : (pa + 1) * S],
                        dq_ps[a][a * 64 : (a + 1) * 64, :],
                        zbc_ab[a][a * 64 : (a + 1) * 64, :],
                        MUL,
                    )

            for p in range(NPAIR):
                # ---- projections: QT2/KT2 [z2, s] ----------------------------
                qt2 = sb2.tile([128, S], BF16, tag="qt2")
                kt2 = sb2.tile([128, S], BF16, tag="kt2")
                for wt, dst in ((wtq, qt2), (wtk, kt2)):
                    pss = [
                        ps_sc.tile([128, 512], F32, tag="ps_sc",
                                   name=f"pj{p}_{dst.tensor.name}_{ch}")
                        for ch in range(NCH)
                    ]
                    for dt in range(ND):
                        j = p * ND + dt
                        for ch in range(NCH):
                            nc.tensor.matmul(
                                pss[ch][:],
                                lhsT=wt[:, j * 128 : (j + 1) * 128],
                                rhs=gt[:, dt * S + ch * 512 : dt * S + ch * 512 + 512],
                                start=(dt == 0),
                                stop=(dt == ND - 1),
                            )
                    for ch in range(NCH):
                        nc.vector.tensor_copy(
                            dst[:, ch * 512 : (ch + 1) * 512], pss[ch][:]
                        )

                # ---- natural-layout transposes: Qraw / K2n [s, z2] -----------
                qraw = sb2.tile([128, S], F16, tag="qraw")
                k2n = sb2.tile([128, S], F16, tag="k2n")
                for src, dst in ((qt2, qraw), (kt2, k2n)):
                    ps = ps_sc.tile([128, S], BF16, tag="ps_sc", name=f"tr{p}_{dst.tensor.name}")
                    for i in range(NQ):
                        nc.tensor.transpose(
                            ps[:, i * 128 : (i + 1) * 128],
                            src[:, i * 128 : (i + 1) * 128],
                            ident_b[:],
                        )
                    nc.vector.tensor_copy(dst[:], ps[:])

                # previous pair's deferred dQ^T burst: emitted after this pair's
                # proj/transposes so the new scores/exps win scheduler priority
                emit_dq_burst()

                # ---- fused scores/exp/accumulate loop ------------------------
                # scores psums are [128,512] (1 bank) and the exp runs per
                # chunk with its own Z cell (accum_out overwrites), combined
                # into zsum2 on DVE.
                zsum2 = sb2.tile([128, 16], F32, tag="zsum2")  # [(q), (head, qb)]
                zsum4 = sb2.tile([128, 32], F32, tag="zsum4")  # [(q), (ch, head, qb)]
                dk_ps = [
                    ps_d.tile([128, S], F32, tag="ps_d", name=f"dk_ps{p}_{a}")
                    for a in range(2)
                ]
                PT_all = pp.tile([128, 2 * NQ * S], F16, tag="PT", bufs=1, name=f"PT{p}")
                PT_r = PT_all[:].rearrange("p (j s) -> p j s", j=2 * NQ)
                for i in range(NQ):
                    # scores blocks [q_i, k] for both heads (row-tiled pairs)
                    # + P blocks + per-chunk Z row-sums
                    P_t = []
                    for a in range(2):
                        c = a * NQ + i
                        pb = pp.tile([128, S], F16, tag="P", name=f"P{p}_{i}_{a}")
                        for ch in range(NCH):
                            ps = ps_sc.tile([128, 512], F32, tag="ps_sc",
                                            name=f"sc{p}_{i}_{a}_{ch}")
                            nc.tensor.matmul(
                                ps[:],
                                lhsT=qt2[a * 64 : (a + 1) * 64, i * 128 : (i + 1) * 128],
                                rhs=kt2[a * 64 : (a + 1) * 64, ch * 512 : (ch + 1) * 512],
                                start=True,
                                stop=True,
                            )
                            nc.scalar.activation(
                                pb[:, ch * 512 : (ch + 1) * 512],
                                ps[:],
                                EXP,
                                scale=float(BETA),
                                accum_out=zsum4[:, ch * 16 + c : ch * 16 + c + 1],
                            )
                        nc.vector.tensor_tensor(
                            zsum2[:, c : c + 1],
                            zsum4[:, c : c + 1],
                            zsum4[:, 16 + c : 16 + c + 1],
                            ADD,
                        )
                        P_t.append(pb)
                    # Qn block = Qraw_i / Z_i
                    q2n_t = sb4.tile([128, 128], F16, tag="q2n", name=f"q2n{p}_{i}")
                    for a in range(2):
                        zq = sb4.tile([128, 1], F32, tag="zq", name=f"zq{p}_{i}_{a}")
                        nc.vector.reciprocal(zq[:], zsum2[:, a * NQ + i : a * NQ + i + 1])
                        nc.vector.tensor_scalar_mul(
                            q2n_t[:, a * 64 : (a + 1) * 64],
                            qraw[:, i * 128 + a * 64 : i * 128 + (a + 1) * 64],
                            zq[:],
                        )
                    # P^T blocks via PE transpose (replaces scoresT recompute).
                    # Emitted BEFORE the dK matmuls: they only depend on the
                    # exp, while dK also waits on the reciprocal/scale chain,
                    # so the in-order PE queue keeps busy.  GPSIMD cannot read
                    # PSUM, so the evacuation copies are split between DVE and
                    # ACT to balance engine load.
                    for a in range(2):
                        ps = ps_sc.tile([128, S], F16, tag="ps_sc", name=f"pt{p}_{i}_{a}")
                        for j in range(NQ):
                            nc.tensor.transpose(
                                ps[:, j * 128 : (j + 1) * 128],
                                P_t[a][:, j * 128 : (j + 1) * 128],
                                ident_h[:],
                            )
                        dst = PT_r[:, a * NQ : (a + 1) * NQ, i * 128 : (i + 1) * 128]
                        src = ps[:].rearrange("p (j c) -> p j c", j=NQ)
                        if a == 0:
                            nc.vector.tensor_copy(dst, src)
                        else:
                            nc.scalar.activation(dst, src, CPY)
                    # dK^T += Qn_i^T P_i (col-tiled pair; each head's
                    # accumulator owns its own psum tile/banks)
                    for a in range(2):
                        for ch in range(NCH):
                            nc.tensor.matmul(
                                dk_ps[a][a * 64 : (a + 1) * 64, ch * 512 : (ch + 1) * 512],
                                lhsT=q2n_t[:, a * 64 : (a + 1) * 64],
                                rhs=P_t[a][:, ch * 512 : (ch + 1) * 512],
                                start=(i == 0),
                                stop=(i == NQ - 1),
                            )

                # ---- Z^-1 broadcast [z2, q] then evacuate accumulators -------
                zinv2 = sb2.tile([128, 16], F32, tag="zinv2")
                nc.vector.reciprocal(zinv2[:], zsum2[:])
                zt_ps = ps_sc.tile([128, 128], F32, tag="ps_sc", name=f"ztp{p}")
                nc.tensor.transpose(zt_ps[0:16, 0:128], zinv2[:], ident[:])
                nc.vector.tensor_copy(ztsb[:], zt_ps[0:16, 0:128])
                nc.sync.dma_start(
                    zrowA[:].rearrange("p (b c) -> p b c", b=NQ), ztsb[0:NQ, :]
                )
                nc.sync.dma_start(
                    zrowB[:].rearrange("p (b c) -> p b c", b=NQ), ztsb[NQ : 2 * NQ, :]
                )
                # partition_broadcast is only correct to base partition 0 ->
                # broadcast each head's Z row across a full tile, read halves.
                zbcA = sb2.tile([128, S], F32, tag="zbcA")
                zbcB = sb2.tile([128, S], F32, tag="zbcB")
                nc.gpsimd.partition_broadcast(zbcA[:], zrowA[:])
                nc.gpsimd.partition_broadcast(zbcB[:], zrowB[:])

                for a in range(2):
                    nc.vector.tensor_copy(
                        dkt2[a * 64 : (a + 1) * 64, p * S : (p + 1) * S],
                        dk_ps[a][a * 64 : (a + 1) * 64, :],
                    )

                # (the dQ^T burst for this pair is emitted lazily -- see
                # emit_dq_burst -- so the next pair's scores/exps get priority)
                pending_dq.append((PT_all, k2n, (zbcA, zbcB), p))

            emit_dq_burst()

            # ---- output projection  sum_h dQ wq + dK wk, then RS -------------
            # Single bf16 ReduceScatter of the full [S, D] partial: rank r of
            # each 4-core batch group receives rows 256r..256r+255 -- exactly
            # the gradient rows of its OWN uploaded x chunk, so the host
            # unshard is a pure reshape.
            rs_in = dram.tile([S, D], BF16, tag="rs_in", name=f"rsi{_rep}")
            rs_out = dram.tile([CHUNK, D], BF16, tag="rs_out", name=f"rso{_rep}")
            for sb in range(NQ):
                pss = [
                    ps_sc.tile([128, 512], F32, tag="ps_sc", name=f"op{sb}_{ch}")
                    for ch in range(NCH)
                ]
                n = 0
                for p in range(NPAIR):
                    for dmat, wmat in ((dqt2, wnq), (dkt2, wnk)):
                        for ch in range(NCH):
                            nc.tensor.matmul(
                                pss[ch][:],
                                lhsT=dmat[:, p * S + sb * 128 : p * S + (sb + 1) * 128],
                                rhs=wmat[:, p * D + ch * 512 : p * D + ch * 512 + 512],
                                start=(n == 0),
                                stop=(n == 2 * NPAIR - 1),
                            )
                        n += 1
                go = sb4.tile([128, S], BF16, tag="go", name=f"go{sb}")
                for ch in range(NCH):
                    nc.vector.tensor_copy(go[:, ch * 512 : (ch + 1) * 512], pss[ch][:])
                nc.sync.dma_start(rs_in[sb * 128 : (sb + 1) * 128, :], go[:])
            nc.gpsimd.collective_compute(
                "ReduceScatter", ADD, replica_groups=GROUPS,
                ins=[rs_in[:]], outs=[rs_out[:]],
            )
            # fused out = pos - acc on the RS'd own-chunk rows
            for m in range(2):
                rsb = sb4.tile([128, D], BF16, tag="rsb", name=f"rsb{m}")
                nc.sync.dma_start(rsb[:], rs_out[m * 128 : (m + 1) * 128, :])
                ob = sb4.tile([128, D], F32, tag="ob", name=f"ob{m}")
                psb = sb2.tile([128, 1], F32, tag="psb", name=f"psb{m}")
                nc.sync.dma_start(psb[:], posc[:, m : m + 1])
                nc.vector.tensor_scalar(
                    ob[:], rsb[:], -1.0, psb[:], MUL, ADD
                )
                nc.sync.dma_start(gout[m * 128 : (m + 1) * 128, :], ob[:])

    nc.compile()
    return nc


def core_inputs(x, wq, wk, core):
    """Per-core input arrays (host-side shard/layout prep, all cheap)."""
    import ml_dtypes

    BF = ml_dtypes.bfloat16
    b, r = core // 4, core % 4
    xck = np.ascontiguousarray(x[b, r * CHUNK : (r + 1) * CHUNK, :]).astype(BF)
    h0 = 4 * (core % 4)
    wqn = np.ascontiguousarray(wq[h0 : h0 + 4].reshape(NPAIR * 128, D)).astype(BF)
    wkn = np.ascontiguousarray(wk[h0 : h0 + 4].reshape(NPAIR * 128, D)).astype(BF)
    pos = np.linspace(-0.5, 0.5, S, dtype=np.float32) * np.float32(POS_SCALE)
    posc = np.ascontiguousarray(
        pos[r * CHUNK : (r + 1) * CHUNK].reshape(2, 128).T
    )
    return {"xc": xck, "wqn": wqn, "wkn": wkn, "posc": posc}


def combine(gouts):
    """Host unshard: each core returns its own chunk's rows -- pure reshape."""
    return np.asarray(gouts, np.float32).reshape(B, S, D)


def _build_persistent(nc):
    """One-time jitted sharded callable over the Bass NEFF (no per-call
    retracing; outputs are fully written by the kernel so no donation)."""
    import jax
    import numpy as _np
    from jax.experimental.shard_map import shard_map
    from jax.sharding import Mesh, NamedSharding, PartitionSpec

    import concourse.mybir as mybir
    from concourse import bass2jax

    bass2jax.install_neuronx_cc_hook()
    partition_name = nc.partition_id_tensor.name if nc.partition_id_tensor else None
    in_names, out_names, out_avals = [], [], []
    for alloc in nc.m.functions[0].allocations:
        if not isinstance(alloc, mybir.MemoryLocationSet):
            continue
        name = alloc.memorylocations[0].name
        if alloc.kind == "ExternalInput":
            if name != partition_name:
                in_names.append(name)
        elif alloc.kind == "ExternalOutput":
            out_names.append(name)
            out_avals.append(
                jax.core.ShapedArray(tuple(alloc.tensor_shape), mybir.dt.np(alloc.dtype))
            )
    n_params = len(in_names)
    all_in_names = list(in_names) + out_names
    if partition_name is not None:
        all_in_names.append(partition_name)

    def _body(*args):
        operands = list(args)
        if partition_name is not None:
            operands.append(bass2jax.partition_id_tensor())
        return tuple(
            bass2jax._bass_exec_p.bind(
                *operands,
                out_avals=tuple(out_avals),
                in_names=tuple(all_in_names),
                out_names=tuple(out_names),
                lowering_input_output_aliases=(),
                sim_require_finite=True,
                sim_require_nnan=True,
                nc=nc,
            )
        )

    devices = jax.devices()[:N_CORES]
    mesh = Mesh(_np.asarray(devices), ("core",))
    spec = PartitionSpec("core")
    sharded = jax.jit(
        shard_map(
            _body,
            mesh=mesh,
            in_specs=(spec,) * (n_params + len(out_names)),
            out_specs=(spec,) * len(out_names),
            check_rep=False,
        ),
        keep_unused=True,
    )
    sh = NamedSharding(mesh, spec)
    zeros = [
        jax.device_put(
            _np.zeros((N_CORES * a.shape[0],) + a.shape[1:], a.dtype), sh
        )
        for a in out_avals
    ]
    return {
        "sharded": sharded,
        "in_names": in_names,
        "out_names": out_names,
        "out_avals": out_avals,
        "sh": sh,
        "zeros": zeros,
        "jax": jax,
    }


def kernel(x, wq, wk):
    x = np.asarray(x, np.float32)
    wq = np.asarray(wq, np.float32)
    wk = np.asarray(wk, np.float32)
    if "nc" not in _CACHE:
        _CACHE["nc"] = build_nc()
    nc = _CACHE["nc"]
    if "pc" not in _CACHE:
        _CACHE["pc"] = _build_persistent(nc)
    pc = _CACHE["pc"]
    jax = pc["jax"]

    in_maps = [core_inputs(x, wq, wk, c) for c in range(N_CORES)]
    concat_in = [
        jax.device_put(
            np.concatenate([np.asarray(m[nm]) for m in in_maps], axis=0), pc["sh"]
        )
        for nm in pc["in_names"]
    ]
    outs = pc["sharded"](*concat_in, *pc["zeros"])
    g = np.asarray(outs[pc["out_names"].index("gout")])
    return combine(g.reshape(N_CORES, CHUNK, D))
